# revision 5
# baseline (speedup 1.0000x reference)
"""Grok1-style attention on 8 trn2 NeuronCores, tensor-parallel over heads.

Sharding (per core c of 8):
  - q heads 4c..4c+3 (512 features), kv head c (128+128 features)
  - w_qkv sharded column-wise (by head), w_o row-wise; partial o_proj
    outputs summed on host (the all-reduce).

Device layout trick: qkv is computed TRANSPOSED (features on partitions,
positions on free axis), so scores (k^T q), probs*V and o_proj chain with
no transposes except 16 cheap PE transposes of V.

Softmax: tanh logit-cap bounds scores to +-30 so exp() cannot overflow ->
no row-max pass. Row sums via ones-vector matmul on the PE (scores are
held k-on-partitions); normalization via a rank-1 broadcast matmul.

Precision: bf16 matmul operands (PE runs fp32 4x slower), fp32 PSUM
accumulation, tanh kept in fp32 (bf16 there would put ~0.06 absolute
error into the exponent).
"""
import numpy as np
from contextlib import ExitStack

import concourse.bass as bass
import concourse.mybir as mybir
import concourse.tile as tile
from concourse import bacc
from concourse.bass_utils import run_bass_kernel_spmd
from concourse.masks import make_identity

T = 2048
D = 4096
HD = 128
HALF = 64
NCORES = 8
HPC = 4                    # q heads per core
QF = HPC * HD              # 512
NF = QF + 2 * HD           # 768 qkv features per core
NCH = D // 128             # 32 contraction chunks
TT = 512                   # t-tile width (matmul moving dim)
NTT = T // TT              # 4
NKT = T // 128             # 16 k-tiles
SCALING = HD ** -0.5
CAP = 30.0
BF = mybir.dt.bfloat16
F32 = mybir.dt.float32


def _emit(nc):
    hT = nc.dram_tensor("hT", [D, T], F32, kind="ExternalInput").ap()
    wq = nc.dram_tensor("wq", [D, NF], F32, kind="ExternalInput").ap()
    wo = nc.dram_tensor("wo", [QF, D], F32, kind="ExternalInput").ap()
    cc = nc.dram_tensor("cc", [HD, T], F32, kind="ExternalInput").ap()
    ss = nc.dram_tensor("ss", [HD, T], F32, kind="ExternalInput").ap()
    out = nc.dram_tensor("out", [T, D], F32, kind="ExternalOutput").ap()

    with tile.TileContext(nc) as tc:
        with ExitStack() as ctx:
            wqp = ctx.enter_context(tc.tile_pool(name="wqp", bufs=1))
            bigp = ctx.enter_context(tc.tile_pool(name="bigp", bufs=2))
            cstp = ctx.enter_context(tc.tile_pool(name="cstp", bufs=1))
            seqp = ctx.enter_context(tc.tile_pool(name="seqp", bufs=1))
            rtp = ctx.enter_context(tc.tile_pool(name="rtp", bufs=3))
            stp = ctx.enter_context(tc.tile_pool(name="stp", bufs=3))
            etp = ctx.enter_context(tc.tile_pool(name="etp", bufs=4))
            smp = ctx.enter_context(tc.tile_pool(name="smp", bufs=2))
            obp = ctx.enter_context(tc.tile_pool(name="obp", bufs=4))
            psp = ctx.enter_context(tc.tile_pool(name="psp", bufs=1, space="PSUM"))

            # ---- constants + resident loads ----
            wq_sb = wqp.tile([128, NCH, NF], BF, tag="wq")
            nc.gpsimd.dma_start(wq_sb[:], wq.rearrange("(c p) f -> p c f", p=128))
            cc_sb = cstp.tile([HD, T], BF, tag="cc")
            ss_sb = cstp.tile([HD, T], BF, tag="ss")
            nc.gpsimd.dma_start(cc_sb[:], cc[:, :])
            nc.gpsimd.dma_start(ss_sb[:], ss[:, :])
            ident = cstp.tile([128, 128], BF, tag="id")
            make_identity(nc, ident[:])
            ones_k = cstp.tile([128, 1], BF, tag="ones_k")
            nc.gpsimd.memset(ones_k[:], 1.0)
            ones_m = cstp.tile([1, 128], BF, tag="ones_m")
            nc.gpsimd.memset(ones_m[:], 1.0)

            qT = [seqp.tile([HD, T], BF, tag=f"q{h}", name=f"qT{h}") for h in range(HPC)]
            kT = seqp.tile([HD, T], BF, tag="kT")
            vT = seqp.tile([HD, T], BF, tag="vT")
            v_sb = seqp.tile([128, T], BF, tag="v_sb")
            attnT = [seqp.tile([HD, T], BF, tag=f"at{h}", name=f"attnT{h}") for h in range(HPC)]

            # ---- phase 1: qkv projection (transposed) + rope ----
            hT_r = hT.rearrange("(c p) t -> p c t", p=128)
            for tt in range(NTT):
                t0 = tt * TT
                h_a = bigp.tile([128, NCH // 2, TT], BF, tag="big", name="h_a")
                nc.gpsimd.dma_start(h_a[:], hT_r[:, 0:NCH // 2, t0:t0 + TT])
                h_b = bigp.tile([128, NCH // 2, TT], BF, tag="big", name="h_b")
                nc.gpsimd.dma_start(h_b[:], hT_r[:, NCH // 2:NCH, t0:t0 + TT])
                ps = [psp.tile([128, TT], F32, tag=f"b{f}", name=f"qkv_ps{f}") for f in range(6)]
                for c in range(NCH):
                    for f in range(6):
                        nc.tensor.matmul(
                            ps[f][:],
                            wq_sb[:, c, f * 128:(f + 1) * 128],
                            (h_a if c < NCH // 2 else h_b)[:, c % (NCH // 2), :],
                            start=(c == 0),
                            stop=(c == NCH - 1),
                        )
                c_t = cc_sb[:, t0:t0 + TT]
                s_t = ss_sb[:, t0:t0 + TT]
                for f in range(5):
                    dst = qT[f] if f < HPC else kT
                    qk_sb = rtp.tile([128, TT], BF, tag="qk_sb")
                    nc.scalar.copy(qk_sb[:], ps[f][:])
                    # rotated copy: rot[0:64]=x2, rot[64:128]=x1 (partition swap via DMA)
                    rot = rtp.tile([128, TT], BF, tag="rot")
                    nc.sync.dma_start(rot[0:HALF, :], qk_sb[HALF:128, :])
                    nc.sync.dma_start(rot[HALF:128, :], qk_sb[0:HALF, :])
                    m1 = rtp.tile([128, TT], BF, tag="m1")
                    nc.vector.tensor_mul(m1[:], qk_sb[:], c_t)
                    m2 = rtp.tile([128, TT], BF, tag="m2")
                    nc.vector.tensor_mul(m2[:], rot[:], s_t)
                    nc.vector.tensor_add(dst[:, t0:t0 + TT], m1[:], m2[:])
                nc.scalar.copy(vT[:, t0:t0 + TT], ps[5][:])

            # ---- phase 1.5: transpose V to [t, d] blocks ----
            for kt in range(NKT):
                k0 = kt * 128
                tp = psp.tile([128, 128], BF, tag=f"b{6 + kt % 2}", name="tp")
                nc.tensor.transpose(tp[:], vT[:, k0:k0 + 128], ident[:])
                nc.vector.tensor_copy(v_sb[:, k0:k0 + 128], tp[:])

            # ---- phase 2: attention per head ----
            for h in range(HPC):
                for qt in range(NTT):
                    q0 = qt * TT
                    a_ps = psp.tile([HD, TT], F32, tag="b0", name="a_ps")
                    d_ps = psp.tile([1, TT], F32, tag="b1", name="d_ps")
                    nkt = 4 * qt + 4
                    for kt in range(nkt):
                        k0 = kt * 128
                        s_ps = psp.tile([128, TT], F32, tag=f"b{2 + kt % 2}", name="s_ps")
                        nc.tensor.matmul(
                            s_ps[:], kT[:, k0:k0 + 128], qT[h][:, q0:q0 + TT],
                            start=True, stop=True,
                        )
                        st = stp.tile([128, TT], F32, tag="st")
                        nc.scalar.activation(
                            st[:], s_ps[:], mybir.ActivationFunctionType.Tanh,
                            scale=SCALING / CAP,
                        )
                        et = etp.tile([128, TT], BF, tag="et")
                        nc.scalar.activation(
                            et[:], st[:], mybir.ActivationFunctionType.Exp,
                            scale=CAP,
                        )
                        m = kt - 4 * qt
                        if m >= 0:
                            # causal: keep where (q0+j) - (k0+i) >= 0
                            nc.gpsimd.affine_select(
                                out=et[:], in_=et[:],
                                pattern=[[1, TT]],
                                compare_op=mybir.AluOpType.is_ge,
                                fill=0.0,
                                base=-(128 * m),
                                channel_multiplier=-1,
                            )
                        last = kt == nkt - 1
                        nc.tensor.matmul(
                            a_ps[:], v_sb[:, k0:k0 + 128], et[:],
                            start=(kt == 0), stop=last,
                        )
                        nc.tensor.matmul(
                            d_ps[:], ones_k[:], et[:],
                            start=(kt == 0), stop=last,
                        )
                    rc = smp.tile([1, TT], F32, tag="rc")
                    nc.vector.reciprocal(rc[:], d_ps[:])
                    rcb = smp.tile([1, TT], BF, tag="rcb")
                    nc.vector.tensor_copy(rcb[:], rc[:])
                    bc_ps = psp.tile([128, TT], F32, tag="b4", name="bc_ps")
                    nc.tensor.matmul(bc_ps[:], ones_m[:], rcb[:], start=True, stop=True)
                    bc_sb = smp.tile([128, TT], F32, tag="bcs")
                    nc.vector.tensor_copy(bc_sb[:], bc_ps[:])
                    nc.vector.tensor_mul(attnT[h][:, q0:q0 + TT], a_ps[:], bc_sb[:])

            # ---- phase 3: o_proj partial ----
            wo_r = wo.rearrange("(c p) n -> p c n", p=128)
            wo_t = []
            for j in range(2):
                w_j = bigp.tile([128, 2, D], BF, tag="big", name=f"wo{j}")
                nc.gpsimd.dma_start(w_j[:], wo_r[:, 2 * j:2 * j + 2, :])
                wo_t.append(w_j)
            for t16 in range(T // 128):
                t0 = t16 * 128
                for half in range(2):
                    pls = [psp.tile([128, TT], F32, tag=f"b{half * 4 + n}", name=f"o_ps{half}{n}") for n in range(4)]
                    for fc in range(HPC):
                        lhsT = attnT[fc][:, t0:t0 + 128]
                        for n in range(4):
                            n0 = (half * 4 + n) * TT
                            nc.tensor.matmul(
                                pls[n][:], lhsT, wo_t[fc // 2][:, fc % 2, n0:n0 + TT],
                                start=(fc == 0), stop=(fc == HPC - 1),
                            )
                    for n in range(4):
                        n0 = (half * 4 + n) * TT
                        ob = obp.tile([128, TT], F32, tag="ob")
                        nc.vector.tensor_copy(ob[:], pls[n][:])
                        nc.sync.dma_start(out[t0:t0 + 128, n0:n0 + TT], ob[:])
    return nc


_CACHE = {}


def _get_nc():
    if "nc" not in _CACHE:
        nc = bacc.Bacc("TRN2", target_bir_lowering=False, debug=False)
        _emit(nc)
        nc.compile()
        _CACHE["nc"] = nc
    return _CACHE["nc"]


def _in_maps(positions, hidden_states, w_qkv, w_o):
    hidden_states = np.asarray(hidden_states, dtype=np.float32)
    w_qkv = np.asarray(w_qkv, dtype=np.float32)
    w_o = np.asarray(w_o, dtype=np.float32)
    pos = np.asarray(positions).astype(np.float64)

    hT = np.ascontiguousarray(hidden_states.T)
    inv_freq = 1.0 / (10000.0 ** (np.arange(HALF, dtype=np.float64) * 2.0 / HD))
    ang = np.outer(inv_freq, pos)                      # [64, T]
    cos = np.cos(ang).astype(np.float32)
    sin = np.sin(ang).astype(np.float32)
    cc = np.ascontiguousarray(np.concatenate([cos, cos], axis=0))   # [128, T]
    ss = np.ascontiguousarray(np.concatenate([-sin, sin], axis=0))  # [128, T]

    in_maps = []
    for c in range(NCORES):
        rows = np.concatenate([
            w_qkv[QF * c:QF * (c + 1)],
            w_qkv[D + HD * c:D + HD * (c + 1)],
            w_qkv[D + HD * NCORES + HD * c:D + HD * NCORES + HD * (c + 1)],
        ], axis=0)                                      # [768, 4096]
        wq_c = np.ascontiguousarray(rows.T)             # [4096, 768]
        wo_c = np.ascontiguousarray(w_o[:, QF * c:QF * (c + 1)].T)  # [512, 4096]
        in_maps.append({"hT": hT, "wq": wq_c, "wo": wo_c, "cc": cc, "ss": ss})
    return in_maps


def run(positions, hidden_states, w_qkv, w_o, trace=False):
    nc = _get_nc()
    in_maps = _in_maps(positions, hidden_states, w_qkv, w_o)
    res = run_bass_kernel_spmd(nc, in_maps, list(range(NCORES)), trace=trace)
    parts = np.stack([res.results[i]["out"] for i in range(NCORES)], axis=0)
    full = parts.sum(axis=0, dtype=np.float64).astype(np.float32)
    return full, res


def kernel(positions, hidden_states, w_qkv, w_o):
    full, _ = run(positions, hidden_states, w_qkv, w_o, trace=False)
    return full
